# revision 1
# baseline (speedup 1.0000x reference)
"""Trainium2 Bass kernel for nn_Attention_86431921864842.

Decode-style attention: B=16 batches, H=16 heads, Sq=16 new tokens,
4096-token KV cache, RoPE-extended 128-dim scores, fused QKV + output
projections.

Sharding: tensor-parallel over heads, 8 cores x 2 heads each.  Each core
receives the full x, its 2-head slice of w_qkv (transposed), its 2-head
column slice of w_o (transposed), and its heads' K/rot/V caches in
device-friendly layouts:

  k2t [32, 128, 4096]  - per (head_local, batch): rows 0:64 = cache_k^T,
                         rows 64:128 = cache_pos_k_rot^T (d on partitions)
  vv  [32, 128, 32, 65] - V cache tiled [tile p=128, n=32 tiles, 64 dims]
                          plus a baked-in ones column (col 64) that makes
                          the PV matmul also produce the softmax denominator.

Device per (b,h): S^T tiles = k2T_chunk.T @ q2T  -> exp -> PV accumulate
(attn^T as stationary operand, [V|1] as moving operand) -> per-query
normalize -> o-proj partial.  Host sums the 8 partial o-proj outputs.
"""

import math
import os
import sys

import numpy as np

for _p in ("/opt/trn_rl_repo",):
    if _p not in sys.path and os.path.isdir(_p):
        sys.path.insert(0, _p)

B = 16
H = 16
SQ = 16
DM = 1024
DH = 64
SKV = 4096
ROPE_BASE = 10000.0
N_CORES = 8
H_PER_CORE = H // N_CORES  # 2
E_PER_CORE = H_PER_CORE * 3 * DH  # 384
D_PER_CORE = H_PER_CORE * DH  # 128
BS = B * SQ  # 256
N_KTILES = SKV // 128  # 32
SCALE = 1.0 / math.sqrt(2 * DH)

_PROGRAM = None  # (nc, in_names, out_name)


def _build_program():
    import concourse.bass as bass
    import concourse.mybir as mybir
    import concourse.tile as tile
    from concourse import bacc

    f32 = mybir.dt.float32
    Exp = mybir.ActivationFunctionType.Exp

    nc = bacc.Bacc(
        "TRN2",
        target_bir_lowering=False,
        debug=False,
        enable_asserts=False,
        num_devices=N_CORES,
    )

    xT_d = nc.dram_tensor("xT", [DM, BS], f32, kind="ExternalInput")
    wq_d = nc.dram_tensor("wqkvT", [DM, E_PER_CORE], f32, kind="ExternalInput")
    wo_d = nc.dram_tensor("woT", [D_PER_CORE, DM], f32, kind="ExternalInput")
    k2t_d = nc.dram_tensor("k2t", [2 * B, 128, SKV], f32, kind="ExternalInput")
    vv_d = nc.dram_tensor("vv", [2 * B, 128, N_KTILES, 65], f32, kind="ExternalInput")
    cos_d = nc.dram_tensor("cosN", [128, 32], f32, kind="ExternalInput")
    sin_d = nc.dram_tensor("sinN", [128, 32], f32, kind="ExternalInput")
    id_d = nc.dram_tensor("ident", [128, 128], f32, kind="ExternalInput")
    out_d = nc.dram_tensor("out", [2, 128, DM], f32, kind="ExternalOutput")

    with tile.TileContext(nc) as tc:
        with (
            tc.tile_pool(name="const", bufs=1) as pc,
            tc.tile_pool(name="head", bufs=1) as ph,
            tc.tile_pool(name="rope", bufs=1) as pr,
            tc.tile_pool(name="k2", bufs=2) as pk,
            tc.tile_pool(name="vc", bufs=2) as pv,
            tc.tile_pool(name="exp", bufs=2) as pe,
            tc.tile_pool(name="small", bufs=2) as ps,
            tc.tile_pool(name="ps_s", bufs=2, space="PSUM") as pss,
            tc.tile_pool(name="ps_o", bufs=2, space="PSUM") as pso,
            tc.tile_pool(name="ps_m", bufs=2, space="PSUM") as psm,
        ):
            # ---- constants ----
            xT_sb = pc.tile([128, 8, BS], f32, tag="xT")
            nc.sync.dma_start(xT_sb[:], xT_d[:].rearrange("(dc p) n -> p dc n", p=128))
            wq_sb = pc.tile([128, 8, E_PER_CORE], f32, tag="wq")
            nc.sync.dma_start(wq_sb[:], wq_d[:].rearrange("(dc p) n -> p dc n", p=128))
            wo_sb = pc.tile([128, DM], f32, tag="wo")
            nc.sync.dma_start(wo_sb[:], wo_d[:])
            cos_sb = pc.tile([128, 32], f32, tag="cos")
            nc.sync.dma_start(cos_sb[:], cos_d[:])
            sin_sb = pc.tile([128, 32], f32, tag="sin")
            nc.sync.dma_start(sin_sb[:], sin_d[:])
            id_sb = pc.tile([128, 128], f32, tag="ident")
            nc.sync.dma_start(id_sb[:], id_d[:])

            # ---- QKV projection: qkv_nat[bs, e_local], bs-chunked ----
            qkv_nat = ph.tile([128, 2, E_PER_CORE], f32, tag="qkv_nat")
            for j in range(2):
                psq = psm.tile([128, 512], f32, tag="misc")
                for dc in range(8):
                    nc.tensor.matmul(
                        psq[:, :E_PER_CORE],
                        lhsT=xT_sb[:, dc, j * 128 : (j + 1) * 128],
                        rhs=wq_sb[:, dc, :],
                        start=(dc == 0),
                        stop=(dc == 7),
                    )
                nc.vector.tensor_copy(qkv_nat[:, j, :], psq[:, :E_PER_CORE])

            # ---- RoPE + transposes per local head ----
            cosb = cos_sb[:].unsqueeze(1).to_broadcast([128, 2, 32])
            sinb = sin_sb[:].unsqueeze(1).to_broadcast([128, 2, 32])
            q2T = []  # per head: [128, 256] (d2, bs)
            k2nT = []  # per head: [128, 256]
            vTh = []  # per head: [64, 256] (dv, bs)
            for hl in range(2):
                base = hl * 3 * DH
                qs = qkv_nat[:, :, base : base + 64]
                ks = qkv_nat[:, :, base + 64 : base + 128]

                q2n = pr.tile([128, 2, 128], f32, tag="q2n")
                k2n = pr.tile([128, 2, 128], f32, tag="k2n")
                t1 = pr.tile([128, 2, 32], f32, tag="t1")
                t2 = pr.tile([128, 2, 32], f32, tag="t2")
                for src, dst in ((qs, q2n), (ks, k2n)):
                    x1 = src[:, :, 0:32]
                    x2 = src[:, :, 32:64]
                    nc.vector.tensor_copy(dst[:, :, 0:64], src)
                    nc.vector.tensor_mul(t1[:], x1, cosb)
                    nc.vector.tensor_mul(t2[:], x2, sinb)
                    nc.vector.tensor_sub(dst[:, :, 64:96], t1[:], t2[:])
                    nc.vector.tensor_mul(t1[:], x1, sinb)
                    nc.vector.tensor_mul(t2[:], x2, cosb)
                    nc.vector.tensor_add(dst[:, :, 96:128], t1[:], t2[:])

                q2T_h = ph.tile([128, BS], f32, tag=f"q2T_{hl}")
                k2nT_h = ph.tile([128, BS], f32, tag=f"k2nT_{hl}")
                vT_h = ph.tile([64, BS], f32, tag=f"vT_{hl}")
                for j in range(2):
                    pt = psm.tile([128, 512], f32, tag="misc")
                    nc.tensor.transpose(pt[:, 0:128], q2n[:, j, :], id_sb[:])
                    nc.vector.tensor_copy(q2T_h[:, j * 128 : (j + 1) * 128], pt[:, 0:128])
                    pt2 = psm.tile([128, 512], f32, tag="misc")
                    nc.tensor.transpose(pt2[:, 0:128], k2n[:, j, :], id_sb[:])
                    nc.vector.tensor_copy(
                        k2nT_h[:, j * 128 : (j + 1) * 128], pt2[:, 0:128]
                    )
                    pt3 = psm.tile([128, 512], f32, tag="misc")
                    nc.tensor.transpose(
                        pt3[0:64, 0:128],
                        qkv_nat[:, j, base + 128 : base + 192],
                        id_sb[:],
                    )
                    nc.vector.tensor_copy(vT_h[:, j * 128 : (j + 1) * 128], pt3[0:64, 0:128])
                q2T.append(q2T_h)
                k2nT.append(k2nT_h)
                vTh.append(vT_h)

            # val_sb[s, b, hl, dv] : normalized attention output (natural)
            val_sb = ph.tile([16, B, 2, 64], f32, tag="val_sb")

            # ---- main loop over (head_local, batch) ----
            for hl in range(2):
                for b in range(B):
                    bh = hl * B + b
                    k2 = pk.tile([128, SKV], f32, tag="k2")
                    nc.sync.dma_start(k2[:], k2t_d[bh])
                    vt = pv.tile([128, N_KTILES, 65], f32, tag="vt")
                    nc.sync.dma_start(vt[:], vv_d[bh])

                    qsl = q2T[hl][:, b * 16 : (b + 1) * 16]

                    ps_sT = pss.tile([128, 544], f32, tag="sT")
                    for i in range(N_KTILES):
                        nc.tensor.matmul(
                            ps_sT[:, i * 16 : (i + 1) * 16],
                            lhsT=k2[:, i * 128 : (i + 1) * 128],
                            rhs=qsl,
                            start=True,
                            stop=True,
                        )
                    nc.tensor.matmul(
                        ps_sT[0:16, 512:528],
                        lhsT=k2nT[hl][:, b * 16 : (b + 1) * 16],
                        rhs=qsl,
                        start=True,
                        stop=True,
                    )

                    expT = pe.tile([128, 528], f32, tag="expT")
                    nc.scalar.activation(
                        expT[:, 0:512], ps_sT[:, 0:512], Exp, scale=SCALE
                    )
                    nc.scalar.activation(
                        expT[0:16, 512:528], ps_sT[0:16, 512:528], Exp, scale=SCALE
                    )

                    # new-token V rows to partitions 0:16 via PE transpose
                    ps_vn = psm.tile([128, 512], f32, tag="misc")
                    nc.tensor.transpose(
                        ps_vn[0:16, 0:64],
                        vTh[hl][:, b * 16 : (b + 1) * 16],
                        id_sb[0:64, 0:64],
                    )
                    vn = ps.tile([16, 65], f32, tag="vn")
                    nc.vector.tensor_copy(vn[:, 0:64], ps_vn[0:16, 0:64])
                    nc.vector.memset(vn[:, 64:65], 1.0)

                    ps_o = pso.tile([16, 65], f32, tag="o")
                    for i in range(N_KTILES):
                        nc.tensor.matmul(
                            ps_o[:],
                            lhsT=expT[:, i * 16 : (i + 1) * 16],
                            rhs=vt[:, i, :],
                            start=(i == 0),
                            stop=False,
                        )
                    nc.tensor.matmul(
                        ps_o[:],
                        lhsT=expT[0:16, 512:528],
                        rhs=vn[:],
                        start=False,
                        stop=True,
                    )

                    rec = ps.tile([16, 1], f32, tag="rec")
                    nc.vector.reciprocal(rec[:], ps_o[:, 64:65])
                    nc.vector.tensor_mul(
                        val_sb[:, b, hl, :],
                        ps_o[:, 0:64],
                        rec[:, 0:1].to_broadcast([16, 64]),
                    )

            # ---- epilogue: transpose val to [d, bs], o-proj, store ----
            valT = ph.tile([128, 2, 128], f32, tag="valT")
            for j in range(2):
                pvt = psm.tile([128, 512], f32, tag="misc")
                for bb in range(8):
                    b = j * 8 + bb
                    nc.tensor.transpose(
                        pvt[:, bb * 16 : (bb + 1) * 16],
                        val_sb[:, b, :, :],
                        id_sb[0:16, 0:16],
                    )
                nc.vector.tensor_copy(valT[:, j, :], pvt[:, 0:128])

            out_sb = ph.tile([128, 2, DM], f32, tag="out_sb")
            for j in range(2):
                for h2 in range(2):
                    po = psm.tile([128, 512], f32, tag="misc")
                    nc.tensor.matmul(
                        po[:],
                        lhsT=valT[:, j, :],
                        rhs=wo_sb[:, h2 * 512 : (h2 + 1) * 512],
                        start=True,
                        stop=True,
                    )
                    nc.vector.tensor_copy(out_sb[:, j, h2 * 512 : (h2 + 1) * 512], po[:])
            for j in range(2):
                nc.sync.dma_start(out_d[j], out_sb[:, j, :])

    nc.compile()
    in_names = ["xT", "wqkvT", "woT", "k2t", "vv", "cosN", "sinN", "ident"]
    return nc, in_names, "out"


def _get_program():
    global _PROGRAM
    if _PROGRAM is None:
        _PROGRAM = _build_program()
    return _PROGRAM


def _prep_inputs(x, w_qkv, w_o, cache_k, cache_v, cache_pos_k_rot):
    """Host-side sharding + layout prep. Returns list of per-core in_maps."""
    f32 = np.float32
    x = np.ascontiguousarray(x, dtype=f32)
    w_qkv = np.ascontiguousarray(w_qkv, dtype=f32)
    w_o = np.ascontiguousarray(w_o, dtype=f32)

    xT = np.ascontiguousarray(x.reshape(BS, DM).T)

    wqkvT = np.ascontiguousarray(w_qkv.T)  # [DM, 3*DM]

    # k2t staging: [core, hl, b, 128, SKV]
    k2t = np.empty((N_CORES, 2, B, 128, SKV), dtype=f32)
    k2t[:, :, :, 0:64, :] = cache_k.reshape(B, N_CORES, 2, SKV, DH).transpose(
        1, 2, 0, 4, 3
    )
    k2t[:, :, :, 64:128, :] = cache_pos_k_rot.reshape(
        B, N_CORES, 2, SKV, DH
    ).transpose(1, 2, 0, 4, 3)

    # v staging: [core, hl, b, p, n, 65]
    vv = np.empty((N_CORES, 2, B, 128, N_KTILES, 65), dtype=f32)
    vv[..., 0:64] = cache_v.reshape(B, N_CORES, 2, N_KTILES, 128, DH).transpose(
        1, 2, 0, 4, 3, 5
    )
    vv[..., 64] = 1.0

    # RoPE tables, f32 math mirroring the reference
    j2 = np.arange(0, DH, 2, dtype=f32)
    inv_freq = (1.0 / (ROPE_BASE ** (j2 / f32(DH)))).astype(f32)
    pos = (SKV + np.arange(SQ)).astype(f32)
    ang = pos[:, None] * inv_freq[None, :]  # [16, 32]
    cosN = np.tile(np.cos(ang).astype(f32), (8, 1))  # [128, 32]
    sinN = np.tile(np.sin(ang).astype(f32), (8, 1))

    ident = np.eye(128, dtype=f32)

    in_maps = []
    for c in range(N_CORES):
        in_maps.append(
            {
                "xT": xT,
                "wqkvT": np.ascontiguousarray(
                    wqkvT[:, c * E_PER_CORE : (c + 1) * E_PER_CORE]
                ),
                "woT": np.ascontiguousarray(
                    w_o[:, c * D_PER_CORE : (c + 1) * D_PER_CORE].T
                ),
                "k2t": k2t[c].reshape(2 * B, 128, SKV),
                "vv": vv[c].reshape(2 * B, 128, N_KTILES, 65),
                "cosN": cosN,
                "sinN": sinN,
                "ident": ident,
            }
        )
    return in_maps


def _run(in_maps, trace=False, trace_kwargs=None):
    from concourse import bass_utils

    nc, in_names, out_name = _get_program()
    kwargs = {}
    if trace:
        kwargs["trace"] = True
        if trace_kwargs:
            kwargs.update(trace_kwargs)
    res = bass_utils.run_bass_kernel_spmd(
        nc, in_maps, core_ids=list(range(N_CORES)), **kwargs
    )
    return res


def kernel(x, w_qkv, w_o, cache_k, cache_v, cache_pos_k_rot, mask=None, **_ignored):
    """Full-input entry point: shards internally across 8 NeuronCores."""
    in_maps = _prep_inputs(x, w_qkv, w_o, cache_k, cache_v, cache_pos_k_rot)
    res = _run(in_maps)
    out = np.zeros((BS, DM), dtype=np.float32)
    for c in range(N_CORES):
        out += res.results[c]["out"].reshape(BS, DM)
    return out.reshape(B, SQ, DM)


# revision 2
# speedup vs baseline: 1.6371x; 1.6371x over previous
"""Trainium2 Bass kernel for nn_Attention_86431921864842.

Decode-style attention: B=16 batches, H=16 heads, Sq=16 new tokens,
4096-token KV cache, RoPE-extended 128-dim scores, fused QKV + output
projections.

Sharding: tensor-parallel over heads, 8 cores x 2 heads each.  Each core
receives the full x, its 2-head slice of w_qkv (transposed), its 2-head
column slice of w_o (transposed), and its heads' K/rot/V caches in
device-friendly layouts:

  k2h/k2l [32, 128, 4096] bf16 - per (head_local, batch): rows 0:64 =
      cache_k^T, rows 64:128 = cache_pos_k_rot^T (d on partitions),
      split into bf16 hi + lo halves (hi + lo == f32 value exactly up to
      ~2^-16 relative, so hi*hi' + hi*lo' + lo*hi' gives f32-grade scores
      at bf16 matmul rates).
  vv [32, 128, 32, 65] f32 - V cache tiled [p=128, n=32 tiles, 64 dims]
      plus a baked-in ones column (col 64) so the PV matmul also produces
      the softmax denominator.

Device per (b,h): S^T tiles = k2T_chunk.T @ q2T -> exp -> PV accumulate
(attn^T stationary, [V|1] moving) -> per-query normalize -> o-proj
partial.  PV for pair i is emitted after S^T for pair i+1 (one-stage
software pipeline) so the Tensor engine never stalls on the exp.  Host
sums the 8 partial o-proj outputs.
"""

import math
import os
import sys

import numpy as np

for _p in ("/opt/trn_rl_repo",):
    if _p not in sys.path and os.path.isdir(_p):
        sys.path.insert(0, _p)

B = 16
H = 16
SQ = 16
DM = 1024
DH = 64
SKV = 4096
ROPE_BASE = 10000.0
N_CORES = 8
H_PER_CORE = H // N_CORES  # 2
E_PER_CORE = H_PER_CORE * 3 * DH  # 384
D_PER_CORE = H_PER_CORE * DH  # 128
BS = B * SQ  # 256
N_KTILES = SKV // 128  # 32
SCALE = 1.0 / math.sqrt(2 * DH)

_PROGRAM = None  # (nc, in_names, out_name)


def _build_program():
    import concourse.bass as bass
    import concourse.mybir as mybir
    import concourse.tile as tile
    from concourse import bacc

    f32 = mybir.dt.float32
    bf16 = mybir.dt.bfloat16
    Exp = mybir.ActivationFunctionType.Exp

    nc = bacc.Bacc(
        "TRN2",
        target_bir_lowering=False,
        debug=False,
        enable_asserts=False,
        num_devices=N_CORES,
    )

    xT_d = nc.dram_tensor("xT", [DM, BS], f32, kind="ExternalInput")
    wq_d = nc.dram_tensor("wqkvT", [DM, E_PER_CORE], f32, kind="ExternalInput")
    wo_d = nc.dram_tensor("woT", [D_PER_CORE, DM], f32, kind="ExternalInput")
    k2h_d = nc.dram_tensor("k2h", [2 * B, 128, SKV], bf16, kind="ExternalInput")
    k2l_d = nc.dram_tensor("k2l", [2 * B, 128, SKV], bf16, kind="ExternalInput")
    vv_d = nc.dram_tensor("vv", [2 * B, 128, N_KTILES, 65], f32, kind="ExternalInput")
    cos_d = nc.dram_tensor("cosN", [128, 32], f32, kind="ExternalInput")
    sin_d = nc.dram_tensor("sinN", [128, 32], f32, kind="ExternalInput")
    id_d = nc.dram_tensor("ident", [128, 128], f32, kind="ExternalInput")
    out_d = nc.dram_tensor("out", [2, 128, DM], f32, kind="ExternalOutput")

    with tile.TileContext(nc) as tc:
        with (
            tc.tile_pool(name="const", bufs=1) as pc,
            tc.tile_pool(name="head", bufs=1) as ph,
            tc.tile_pool(name="rope", bufs=1) as pr,
            tc.tile_pool(name="k2", bufs=2) as pk,
            tc.tile_pool(name="vc", bufs=3) as pv,
            tc.tile_pool(name="exp", bufs=2) as pe,
            tc.tile_pool(name="small", bufs=2) as ps,
            tc.tile_pool(name="ps_s", bufs=2, space="PSUM") as pss,
            tc.tile_pool(name="ps_o", bufs=2, space="PSUM") as pso,
            tc.tile_pool(name="ps_m", bufs=2, space="PSUM") as psm,
        ):
            # ---- constants ----
            xT_sb = pc.tile([128, 8, BS], f32, tag="xT")
            nc.sync.dma_start(xT_sb[:], xT_d[:].rearrange("(dc p) n -> p dc n", p=128))
            wq_sb = pc.tile([128, 8, E_PER_CORE], f32, tag="wq")
            nc.sync.dma_start(wq_sb[:], wq_d[:].rearrange("(dc p) n -> p dc n", p=128))
            wo_sb = pc.tile([128, DM], f32, tag="wo")
            nc.sync.dma_start(wo_sb[:], wo_d[:])
            cos_sb = pc.tile([128, 32], f32, tag="cos")
            nc.sync.dma_start(cos_sb[:], cos_d[:])
            sin_sb = pc.tile([128, 32], f32, tag="sin")
            nc.sync.dma_start(sin_sb[:], sin_d[:])
            id_sb = pc.tile([128, 128], f32, tag="ident")
            nc.sync.dma_start(id_sb[:], id_d[:])

            # ---- QKV projection: qkv_nat[bs, e_local], bs-chunked ----
            qkv_nat = ph.tile([128, 2, E_PER_CORE], f32, tag="qkv_nat")
            for j in range(2):
                psq = psm.tile([128, 512], f32, tag="misc")
                for dc in range(8):
                    nc.tensor.matmul(
                        psq[:, :E_PER_CORE],
                        lhsT=xT_sb[:, dc, j * 128 : (j + 1) * 128],
                        rhs=wq_sb[:, dc, :],
                        start=(dc == 0),
                        stop=(dc == 7),
                    )
                nc.vector.tensor_copy(qkv_nat[:, j, :], psq[:, :E_PER_CORE])

            # ---- RoPE + transposes per local head ----
            cosb = cos_sb[:].unsqueeze(1).to_broadcast([128, 2, 32])
            sinb = sin_sb[:].unsqueeze(1).to_broadcast([128, 2, 32])
            q2T = []  # per head: [128, 256] f32 (d2, bs)
            q2Th = []  # bf16 hi
            q2Tl = []  # bf16 lo
            k2nT = []  # per head: [128, 256] f32
            vTh = []  # per head: [64, 256] f32 (dv, bs)
            for hl in range(2):
                base = hl * 3 * DH
                qs = qkv_nat[:, :, base : base + 64]
                ks = qkv_nat[:, :, base + 64 : base + 128]

                q2n = pr.tile([128, 2, 128], f32, tag="q2n")
                k2n = pr.tile([128, 2, 128], f32, tag="k2n")
                t1 = pr.tile([128, 2, 32], f32, tag="t1")
                t2 = pr.tile([128, 2, 32], f32, tag="t2")
                for src, dst in ((qs, q2n), (ks, k2n)):
                    x1 = src[:, :, 0:32]
                    x2 = src[:, :, 32:64]
                    nc.vector.tensor_copy(dst[:, :, 0:64], src)
                    nc.vector.tensor_mul(t1[:], x1, cosb)
                    nc.vector.tensor_mul(t2[:], x2, sinb)
                    nc.vector.tensor_sub(dst[:, :, 64:96], t1[:], t2[:])
                    nc.vector.tensor_mul(t1[:], x1, sinb)
                    nc.vector.tensor_mul(t2[:], x2, cosb)
                    nc.vector.tensor_add(dst[:, :, 96:128], t1[:], t2[:])

                q2T_h = ph.tile([128, BS], f32, tag=f"q2T_{hl}")
                k2nT_h = ph.tile([128, BS], f32, tag=f"k2nT_{hl}")
                vT_h = ph.tile([64, BS], f32, tag=f"vT_{hl}")
                for j in range(2):
                    pt = psm.tile([128, 512], f32, tag="misc")
                    nc.tensor.transpose(pt[:, 0:128], q2n[:, j, :], id_sb[:])
                    nc.vector.tensor_copy(q2T_h[:, j * 128 : (j + 1) * 128], pt[:, 0:128])
                    pt2 = psm.tile([128, 512], f32, tag="misc")
                    nc.tensor.transpose(pt2[:, 0:128], k2n[:, j, :], id_sb[:])
                    nc.vector.tensor_copy(
                        k2nT_h[:, j * 128 : (j + 1) * 128], pt2[:, 0:128]
                    )
                    pt3 = psm.tile([128, 512], f32, tag="misc")
                    nc.tensor.transpose(
                        pt3[0:64, 0:128],
                        qkv_nat[:, j, base + 128 : base + 192],
                        id_sb[:],
                    )
                    nc.vector.tensor_copy(vT_h[:, j * 128 : (j + 1) * 128], pt3[0:64, 0:128])

                # bf16 hi/lo split of q2T for the score matmuls
                q2h = ph.tile([128, BS], bf16, tag=f"q2h_{hl}")
                q2l = ph.tile([128, BS], bf16, tag=f"q2l_{hl}")
                q2w = pr.tile([128, BS], f32, tag="q2w")
                nc.vector.tensor_copy(q2h[:], q2T_h[:])  # round to bf16
                nc.vector.tensor_copy(q2w[:], q2h[:])  # widen back
                nc.vector.tensor_sub(q2w[:], q2T_h[:], q2w[:])  # residual
                nc.vector.tensor_copy(q2l[:], q2w[:])  # round residual

                q2T.append(q2T_h)
                q2Th.append(q2h)
                q2Tl.append(q2l)
                k2nT.append(k2nT_h)
                vTh.append(vT_h)

            # ---- new-token V rows, pre-transposed to [s, (hl,b), 65] ----
            vn_all = ph.tile([16, 2, B, 65], f32, tag="vn_all")
            nc.vector.memset(vn_all[:, :, :, 64:65], 1.0)
            for hl in range(2):
                for b in range(B):
                    pvn = psm.tile([128, 512], f32, tag="misc")
                    nc.tensor.transpose(
                        pvn[0:16, 0:64],
                        vTh[hl][:, b * 16 : (b + 1) * 16],
                        id_sb[0:64, 0:64],
                    )
                    nc.vector.tensor_copy(vn_all[:, hl, b, 0:64], pvn[0:16, 0:64])

            # val_sb[s, b, hl, dv] : normalized attention output (natural)
            val_sb = ph.tile([16, B, 2, 64], f32, tag="val_sb")

            # ---- main loop over (head_local, batch), PV pipelined 1 back ----
            def emit_pv(state):
                hl, b, expT, vt = state
                ps_o = pso.tile([16, 65], f32, tag="o")
                for i in range(N_KTILES):
                    nc.tensor.matmul(
                        ps_o[:],
                        lhsT=expT[:, i * 16 : (i + 1) * 16],
                        rhs=vt[:, i, :],
                        start=(i == 0),
                        stop=False,
                    )
                nc.tensor.matmul(
                    ps_o[:],
                    lhsT=expT[0:16, 512:528],
                    rhs=vn_all[:, hl, b, :],
                    start=False,
                    stop=True,
                )
                rec = ps.tile([16, 1], f32, tag="rec")
                nc.vector.reciprocal(rec[:], ps_o[:, 64:65])
                nc.vector.tensor_mul(
                    val_sb[:, b, hl, :],
                    ps_o[:, 0:64],
                    rec[:, 0:1].to_broadcast([16, 64]),
                )

            pending = None
            for hl in range(2):
                for b in range(B):
                    bh = hl * B + b
                    k2h_t = pk.tile([128, SKV], bf16, tag="k2h")
                    nc.sync.dma_start(k2h_t[:], k2h_d[bh])
                    k2l_t = pk.tile([128, SKV], bf16, tag="k2l")
                    nc.sync.dma_start(k2l_t[:], k2l_d[bh])
                    vt = pv.tile([128, N_KTILES, 65], f32, tag="vt")
                    nc.sync.dma_start(vt[:], vv_d[bh])

                    qh = q2Th[hl][:, b * 16 : (b + 1) * 16]
                    ql = q2Tl[hl][:, b * 16 : (b + 1) * 16]

                    ps_sT = pss.tile([128, 544], f32, tag="sT")
                    for i in range(N_KTILES):
                        o = ps_sT[:, i * 16 : (i + 1) * 16]
                        kh = k2h_t[:, i * 128 : (i + 1) * 128]
                        kl = k2l_t[:, i * 128 : (i + 1) * 128]
                        nc.tensor.matmul(o, lhsT=kh, rhs=qh, start=True, stop=False)
                        nc.tensor.matmul(o, lhsT=kh, rhs=ql, start=False, stop=False)
                        nc.tensor.matmul(o, lhsT=kl, rhs=qh, start=False, stop=True)
                    nc.tensor.matmul(
                        ps_sT[0:16, 512:528],
                        lhsT=k2nT[hl][:, b * 16 : (b + 1) * 16],
                        rhs=q2T[hl][:, b * 16 : (b + 1) * 16],
                        start=True,
                        stop=True,
                    )

                    expT = pe.tile([128, 528], f32, tag="expT")
                    nc.scalar.activation(
                        expT[:, 0:512], ps_sT[:, 0:512], Exp, scale=SCALE
                    )
                    nc.scalar.activation(
                        expT[0:16, 512:528], ps_sT[0:16, 512:528], Exp, scale=SCALE
                    )

                    if pending is not None:
                        emit_pv(pending)
                    pending = (hl, b, expT, vt)
            emit_pv(pending)

            # ---- epilogue: transpose val to [d, bs], o-proj, store ----
            valT = ph.tile([128, 2, 128], f32, tag="valT")
            for j in range(2):
                pvt = psm.tile([128, 512], f32, tag="misc")
                for bb in range(8):
                    b = j * 8 + bb
                    nc.tensor.transpose(
                        pvt[:, bb * 16 : (bb + 1) * 16],
                        val_sb[:, b, :, :],
                        id_sb[0:16, 0:16],
                    )
                nc.vector.tensor_copy(valT[:, j, :], pvt[:, 0:128])

            out_sb = ph.tile([128, 2, DM], f32, tag="out_sb")
            for j in range(2):
                for h2 in range(2):
                    po = psm.tile([128, 512], f32, tag="misc")
                    nc.tensor.matmul(
                        po[:],
                        lhsT=valT[:, j, :],
                        rhs=wo_sb[:, h2 * 512 : (h2 + 1) * 512],
                        start=True,
                        stop=True,
                    )
                    nc.vector.tensor_copy(out_sb[:, j, h2 * 512 : (h2 + 1) * 512], po[:])
            for j in range(2):
                nc.sync.dma_start(out_d[j], out_sb[:, j, :])

    nc.compile()
    in_names = ["xT", "wqkvT", "woT", "k2h", "k2l", "vv", "cosN", "sinN", "ident"]
    return nc, in_names, "out"


def _get_program():
    global _PROGRAM
    if _PROGRAM is None:
        _PROGRAM = _build_program()
    return _PROGRAM


def _prep_inputs(x, w_qkv, w_o, cache_k, cache_v, cache_pos_k_rot):
    """Host-side sharding + layout prep. Returns list of per-core in_maps."""
    import ml_dtypes

    f32 = np.float32
    bf16 = ml_dtypes.bfloat16
    x = np.ascontiguousarray(x, dtype=f32)
    w_qkv = np.ascontiguousarray(w_qkv, dtype=f32)
    w_o = np.ascontiguousarray(w_o, dtype=f32)

    xT = np.ascontiguousarray(x.reshape(BS, DM).T)

    wqkvT = np.ascontiguousarray(w_qkv.T)  # [DM, 3*DM]

    # k2 staging: [core, hl, b, 128, SKV] f32, then bf16 hi/lo split
    k2t = np.empty((N_CORES, 2, B, 128, SKV), dtype=f32)
    k2t[:, :, :, 0:64, :] = cache_k.reshape(B, N_CORES, 2, SKV, DH).transpose(
        1, 2, 0, 4, 3
    )
    k2t[:, :, :, 64:128, :] = cache_pos_k_rot.reshape(
        B, N_CORES, 2, SKV, DH
    ).transpose(1, 2, 0, 4, 3)
    k2h = k2t.astype(bf16)
    k2l = (k2t - k2h.astype(f32)).astype(bf16)
    del k2t

    # v staging: [core, hl, b, p, n, 65]
    vv = np.empty((N_CORES, 2, B, 128, N_KTILES, 65), dtype=f32)
    vv[..., 0:64] = cache_v.reshape(B, N_CORES, 2, N_KTILES, 128, DH).transpose(
        1, 2, 0, 4, 3, 5
    )
    vv[..., 64] = 1.0

    # RoPE tables, f32 math mirroring the reference
    j2 = np.arange(0, DH, 2, dtype=f32)
    inv_freq = (1.0 / (ROPE_BASE ** (j2 / f32(DH)))).astype(f32)
    pos = (SKV + np.arange(SQ)).astype(f32)
    ang = pos[:, None] * inv_freq[None, :]  # [16, 32]
    cosN = np.tile(np.cos(ang).astype(f32), (8, 1))  # [128, 32]
    sinN = np.tile(np.sin(ang).astype(f32), (8, 1))

    ident = np.eye(128, dtype=f32)

    in_maps = []
    for c in range(N_CORES):
        in_maps.append(
            {
                "xT": xT,
                "wqkvT": np.ascontiguousarray(
                    wqkvT[:, c * E_PER_CORE : (c + 1) * E_PER_CORE]
                ),
                "woT": np.ascontiguousarray(
                    w_o[:, c * D_PER_CORE : (c + 1) * D_PER_CORE].T
                ),
                "k2h": k2h[c].reshape(2 * B, 128, SKV),
                "k2l": k2l[c].reshape(2 * B, 128, SKV),
                "vv": vv[c].reshape(2 * B, 128, N_KTILES, 65),
                "cosN": cosN,
                "sinN": sinN,
                "ident": ident,
            }
        )
    return in_maps


def _run(in_maps, trace=False, trace_kwargs=None):
    from concourse import bass_utils

    nc, in_names, out_name = _get_program()
    kwargs = {}
    if trace:
        kwargs["trace"] = True
        if trace_kwargs:
            kwargs.update(trace_kwargs)
    res = bass_utils.run_bass_kernel_spmd(
        nc, in_maps, core_ids=list(range(N_CORES)), **kwargs
    )
    return res


def kernel(x, w_qkv, w_o, cache_k, cache_v, cache_pos_k_rot, mask=None, **_ignored):
    """Full-input entry point: shards internally across 8 NeuronCores."""
    in_maps = _prep_inputs(x, w_qkv, w_o, cache_k, cache_v, cache_pos_k_rot)
    res = _run(in_maps)
    out = np.zeros((BS, DM), dtype=np.float32)
    for c in range(N_CORES):
        out += res.results[c]["out"].reshape(BS, DM)
    return out.reshape(B, SQ, DM)
